# revision 7
# baseline (speedup 1.0000x reference)
"""Trainium2 Bass kernel for nn_AFRM_48636209660262.

Conv-BiLSTM autoencoder: 4x strided conv encoder -> channel-split BiLSTM ->
1x1 conv ffwd -> 4x conv_transpose decoder -> gamma*h + x.

Strategy: pure data parallelism over 8 NeuronCores (4 samples each, no
collectives). All activations channel-major [C_chunk(128), B, H, W] with
zero-padded spatial borders; convs are per-tap matmuls accumulated in PSUM
(lhsT = W[Cin_chunk, Cout_chunk], rhs = strided activation taps). BN scale is
folded into conv weights on the host, bias applied in the PSUM->SBUF
evacuation on the Scalar engine. conv_transpose is decomposed into 4 parity
classes x 4 taps. The LSTM runs batch-major with both directions packed into
one [36, *] tile (fwd rows 0-3, rvs rows 32-35 - both legal partition
bases); z_x is precomputed for all timesteps into [16, 4096] tiles (rvs with
reversed step order) and re-staged per round with small SBUF->SBUF DMAs; the
recurrent h@U uses h^T tiles produced by small PE transposes. Matmul
operands are bf16 (f32 PSUM accumulation); the residual add and the final
output stay f32.
"""
import numpy as np
import ml_dtypes

import concourse.mybir as mybir
import concourse.tile as tile
from concourse import bacc
from concourse.bass_utils import run_bass_kernel_spmd
from concourse.masks import make_identity

AF = mybir.ActivationFunctionType
BF16 = mybir.dt.bfloat16
F32 = mybir.dt.float32
NPBF = ml_dtypes.bfloat16

N_CORES = 8
B = 4           # batch per core
C = 256
BN_EPS = 1e-3

_CACHE: dict = {}


def _stepped(start, count, step):
    return slice(start, start + step * (count - 1) + 1, step)


def _build(gamma_nonneg=True, use_bias=False, dbg=None):
    nc = bacc.Bacc("TRN2", target_bir_lowering=False, debug=False,
                   num_devices=N_CORES)

    xin = nc.dram_tensor("xin", [2, 128, B, 66, 66], BF16, kind="ExternalInput").ap()
    xres = nc.dram_tensor("xres", [2, 128, B, 64, 64], F32, kind="ExternalInput").ap()
    wenc = nc.dram_tensor("wenc", [4, 2, 128, 16, 2, 128], BF16, kind="ExternalInput").ap()
    wdec = nc.dram_tensor("wdec", [4, 2, 128, 16, 2, 128], BF16, kind="ExternalInput").ap()
    bconv = nc.dram_tensor("bconv", [128, 20], F32, kind="ExternalInput").ap()
    # LSTM mats: [Wf, Wr, Uf, Ur] x [kc, row, 4096]
    wl = nc.dram_tensor("wl", [4, 8, 128, 4096], BF16, kind="ExternalInput").ap()
    bl = nc.dram_tensor("bl", [2, 16, 4096], BF16, kind="ExternalInput").ap()
    wff = nc.dram_tensor("wff", [128, 4, 2, 128], BF16, kind="ExternalInput").ap()
    out = nc.dram_tensor("out", [2, 128, B, 64, 64], F32, kind="ExternalOutput").ap()

    dbg_ap = None
    dbg_shapes = {
        'l1': [2, 128, B, 34, 34], 'l2': [2, 128, B, 18, 18],
        'l3': [2, 128, B, 10, 10], 'enc4': [2, 128, B * 16],
        'seqT': [128, 8, 16], 'hs': [2, 128, 8, 4, B],
        'd0': [2, 128, B, 6, 6], 'd1': [2, 128, B, 10, 10],
        'd2': [2, 128, B, 18, 18], 'd3': [2, 128, B, 34, 34],
    }
    if dbg is not None:
        dbg_ap = nc.dram_tensor("dbg", dbg_shapes[dbg], BF16,
                                kind="ExternalOutput").ap()

    with tile.TileContext(nc) as tc:
        _trace(nc, tc, xin, xres, wenc, wdec, bconv, wl, bl, wff, out,
               gamma_nonneg, use_bias, dbg, dbg_ap)
    nc.compile()
    return nc


def _trace(nc, tc, xin, xres, wenc, wdec, bconv, wl, bl, wff, out,
           gamma_nonneg, use_bias, dbg, dbg_ap):
    from contextlib import ExitStack

    def memset_border(t, Hp):
        nc.vector.memset(t[:, :, 0, :], 0.0)
        nc.vector.memset(t[:, :, Hp - 1, :], 0.0)
        nc.vector.memset(t[:, :, :, 0], 0.0)
        nc.vector.memset(t[:, :, :, Hp - 1], 0.0)

    # decoder parity taps: out[2m+p] <- pairs (di, k)
    ROW_TAPS = {0: [(-1, 0), (0, 2)], 1: [(0, 1), (1, 3)]}

    with ExitStack() as top:
        persist = top.enter_context(tc.tile_pool(name="persist", bufs=1))

        bias_sb = persist.tile([128, 20], F32)
        nc.sync.dma_start(bias_sb[:], bconv[:])
        ident = persist.tile([128, 128], BF16)
        make_identity(nc, ident[:])
        H = [persist.tile([128, 8, 4, B], BF16, name=f"hT{d}") for d in range(2)]
        enc4 = [[persist.tile([128, 4, 4], BF16, name=f"enc4_{kc}_{b}")
                 for b in range(B)] for kc in range(2)]
        d0 = [persist.tile([128, B, 6, 6], BF16, name=f"d0_{mc}") for mc in range(2)]
        wffsb = persist.tile([128, 4, 2, 128], BF16)
        nc.sync.dma_start(wffsb[:], wff[:])
        for mc in range(2):
            memset_border(d0[mc], 6)

        dram = top.enter_context(tc.tile_pool(name="dram", bufs=1, space="DRAM"))
        esc2 = [[dram.tile([1024], BF16, name=f"esc_{sv}_{b}")
                 for b in range(B)] for sv in range(4)]  # [l] per (s,b)

        # lwa pool spans encoder+lstm: prefetch Wf during encoder; Uf reuses
        with tc.tile_pool(name="lwa", bufs=1) as lwp:
            WLf = [lwp.tile([128, 4096], BF16, tag="lwa", bufs=8,
                            name=f"wf_{kc}") for kc in range(8)]

            # ================= encoder =================
            with tc.tile_pool(name="encp", bufs=1) as ep, \
                 tc.tile_pool(name="encps", bufs=1, space="PSUM") as pp:
                # priority order: first x slices + layer-1 weights, then the
                # rest of the encoder weights, then the LSTM Wf prefetch
                wts = []
                for l in range(4):
                    wts.append([ep.tile([128, 16, 2, 128], BF16, tag="cw",
                                        bufs=4, name=f"wenc{l}_{kc}")
                                for kc in range(2)])
                # x tiles split per (kc, b, row-half) for earliest start
                xt = [[[ep.tile([128, 34, 66], BF16, tag="xcm", bufs=16,
                                name=f"x_{kc}_{b}_{hf}") for hf in range(2)]
                       for b in range(B)] for kc in range(2)]
                for kc in range(2):
                    nc.sync.dma_start(xt[kc][0][0][:], xin[kc, :, 0, 0:34, :])
                for kc in range(2):
                    nc.sync.dma_start(wts[0][kc][:], wenc[0, kc])
                for b in range(B):
                    for hf in range(2):
                        for kc in range(2):
                            if b == 0 and hf == 0:
                                continue
                            nc.sync.dma_start(xt[kc][b][hf][:],
                                              xin[kc, :, b, 32 * hf:32 * hf + 34, :])
                for l in range(1, 4):
                    for kc in range(2):
                        nc.sync.dma_start(wts[l][kc][:], wenc[l, kc])
                for kc in range(8):
                    nc.sync.dma_start(WLf[kc][:], wl[0, kc])

                l1 = [ep.tile([128, B, 34, 34], BF16, tag="echain", bufs=4,
                              name=f"l1_{mc}") for mc in range(2)]
                l2 = [ep.tile([128, B, 18, 18], BF16, tag="echain", bufs=4,
                              name=f"l2_{mc}") for mc in range(2)]
                l3 = [ep.tile([128, B, 10, 10], BF16, tag="echain", bufs=4,
                              name=f"l3_{mc}") for mc in range(2)]
                for mc in range(2):
                    memset_border(l1[mc], 34)
                    memset_border(l2[mc], 18)
                    memset_border(l3[mc], 10)

                # L1: per (b, row-half) groups, b outer for earliest start
                for b in range(B):
                    for hf, oh0 in ((0, 0), (1, 16)):
                        for mc in range(2):
                            ps = pp.tile([128, 512], F32, tag="cps", bufs=6)
                            i = 0
                            for kc in range(2):
                                for t in range(16):
                                    kh, kw = t // 4, t % 4
                                    rhs = xt[kc][b][hf][:,
                                                        _stepped(kh, 16, 2),
                                                        _stepped(kw, 32, 2)]
                                    nc.tensor.matmul(ps[:], wts[0][kc][:, t, mc, :],
                                                     rhs, start=(i == 0), stop=(i == 31))
                                    i += 1
                            nc.scalar.activation(
                                l1[mc][:, b, 1 + oh0:17 + oh0, 1:33], ps[:],
                                AF.Relu, bias=bias_sb[:, mc:mc + 1])

                def enc_layer(wt, act_in, act_out, Hin, bias_idx):
                    OH = Hin // 2
                    groups = [(oh0, 8) for oh0 in (0, 8)] if OH == 16 else [(0, OH)]
                    for mc in range(2):
                        for (oh0, ohn) in groups:
                            ps = pp.tile([128, B * ohn * OH], F32, tag="cps", bufs=6)
                            i = 0
                            for kc in range(2):
                                for t in range(16):
                                    kh, kw = t // 4, t % 4
                                    rhs = act_in[kc][:, :,
                                                     _stepped(kh + 2 * oh0, ohn, 2),
                                                     _stepped(kw, OH, 2)]
                                    nc.tensor.matmul(ps[:], wt[kc][:, t, mc, :],
                                                     rhs, start=(i == 0), stop=(i == 31))
                                    i += 1
                            if act_out is None:
                                psv = ps.rearrange("p (b h w) -> p b h w", b=B, h=4)
                                for b in range(B):
                                    nc.scalar.activation(
                                        enc4[mc][b].rearrange("p h w -> p (h w)"),
                                        psv[:, b], AF.Relu,
                                        bias=bias_sb[:, bias_idx + mc:bias_idx + mc + 1])
                            else:
                                nc.scalar.activation(
                                    act_out[mc][:, :, 1 + oh0:1 + oh0 + ohn, 1:1 + OH],
                                    ps[:], AF.Relu,
                                    bias=bias_sb[:, bias_idx + mc:bias_idx + mc + 1])

                enc_layer(wts[1], l1, l2, 32, 2)
                enc_layer(wts[2], l2, l3, 16, 4)
                enc_layer(wts[3], l3, None, 8, 6)

                if dbg in ('l1', 'l2', 'l3'):
                    src = {'l1': l1, 'l2': l2, 'l3': l3}[dbg]
                    for mc in range(2):
                        nc.sync.dma_start(dbg_ap[mc], src[mc][:])

            # enc4 -> l-major DRAM bounce: l = hh*256 + ww*64 + cc, s=2kc+shi
            for kc in range(2):
                for shi in range(2):
                    sv = 2 * kc + shi
                    for b in range(B):
                        src = enc4[kc][b][shi * 64:(shi + 1) * 64, :, :] \
                            .rearrange("cc hh ww -> cc (hh ww)")
                        dst = esc2[sv][b].rearrange("(hw cc) -> cc hw", cc=64)
                        nc.sync.dma_start(dst, src)
            if dbg == 'enc4':
                for kc in range(2):
                    for b in range(B):
                        nc.sync.dma_start(
                            dbg_ap[kc, :, b * 16:(b + 1) * 16],
                            enc4[kc][b].rearrange("p h w -> p (h w)"))

            # ================= LSTM =================
            with tc.tile_pool(name="lstmp", bufs=1) as lp, \
                 tc.tile_pool(name="lstmps", bufs=1, space="PSUM") as lps:
                # seqT [p, j, (s,b)] straight from the l-major bounce
                seqT = lp.tile([128, 8, 16], BF16)
                seqTr = lp.tile([128, 8, 16], BF16)
                for sv in range(4):
                    for b in range(B):
                        src = esc2[sv][b].rearrange("(j p) -> p j", j=8)
                        nc.sync.dma_start(seqT[:, :, sv * 4 + b], src)
                        nc.sync.dma_start(seqTr[:, :, (3 - sv) * 4 + b], src)
                if dbg == 'seqT':
                    nc.sync.dma_start(dbg_ap[:], seqT[:])

                WLr = [lp.tile([128, 4096], BF16, tag="lwb", bufs=8,
                               name=f"wr_{kc}") for kc in range(8)]
                for kc in range(8):
                    nc.sync.dma_start(WLr[kc][:], wl[1, kc])

                # ---- z_x for all steps; rvs rows are step-reversed so that
                # round t always reads rows [4t, 4t+4)
                zxe = []
                for d, sq, WT in ((0, seqT, WLf), (1, seqTr, WLr)):
                    blt = None
                    if use_bias:
                        blt = lp.tile([16, 4096], BF16, tag="zxj", bufs=4,
                                      name=f"bl{d}")
                        nc.sync.dma_start(blt[:], bl[d])
                    zx = lp.tile([16, 4096], BF16, tag="zxj", bufs=4,
                                 name=f"zxe{d}")
                    for q in range(4):
                        ps = lps.tile([16, 1024], F32, tag="pz", bufs=2)
                        for kc in range(8):
                            for nb in range(2):
                                nc.tensor.matmul(
                                    ps[:, nb * 512:(nb + 1) * 512],
                                    sq[:, kc, :],
                                    WT[kc][:, q * 1024 + nb * 512:
                                           q * 1024 + (nb + 1) * 512],
                                    start=(kc == 0), stop=(kc == 7))
                        if use_bias:
                            bsl = blt[:, q * 1024:(q + 1) * 1024]
                            if d == 1:  # bias rows follow the reversed order too
                                bsl = blt[:, q * 1024:(q + 1) * 1024]
                            nc.vector.tensor_add(zx[:, q * 1024:(q + 1) * 1024],
                                                 ps[:], bsl)
                        else:
                            nc.vector.tensor_copy(zx[:, q * 1024:(q + 1) * 1024],
                                                  ps[:])
                    zxe.append(zx)

                # ---- stage all rounds' z slices early (SBUF->SBUF)
                zst = []
                for t in range(4):
                    z = lp.tile([36, 4096], BF16, tag="zxj", bufs=4,
                                name=f"zst{t}")
                    nc.sync.dma_start(z[0:4, :], zxe[0][4 * t:4 * t + 4, :])
                    nc.sync.dma_start(z[32:36, :], zxe[1][4 * t:4 * t + 4, :])
                    zst.append(z)

                # U matrices reuse the W slots
                ULf = [lwp.tile([128, 4096], BF16, tag="lwa", bufs=8,
                                name=f"uf_{kc}") for kc in range(8)]
                for kc in range(8):
                    nc.sync.dma_start(ULf[kc][:], wl[2, kc])
                ULr = [lp.tile([128, 4096], BF16, tag="lwb", bufs=8,
                               name=f"ur_{kc}") for kc in range(8)]
                for kc in range(8):
                    nc.sync.dma_start(ULr[kc][:], wl[3, kc])
                UL = [ULf, ULr]

                # ---- recurrence: joint [36, *], fwd rows 0-3, rvs rows 32-35
                c_prev = None
                for t in range(4):
                    s_of = {0: t, 1: 3 - t}
                    zsums = []
                    if t > 0:
                        for q in range(4):
                            pz = lps.tile([36, 1024], F32, tag="pz", bufs=2)
                            for d, base in ((0, 0), (1, 32)):
                                sp = s_of[d] + (1 if d else -1)
                                for kc in range(8):
                                    for nb in range(2):
                                        nc.tensor.matmul(
                                            pz[base:base + 4,
                                               nb * 512:(nb + 1) * 512],
                                            H[d][:, kc, sp, :],
                                            UL[d][kc][:, q * 1024 + nb * 512:
                                                      q * 1024 + (nb + 1) * 512],
                                            start=(kc == 0), stop=(kc == 7))
                            zs = lp.tile([36, 1024], F32, tag="ltmp", bufs=6,
                                         name=f"zs{t}_{q}")
                            nc.vector.tensor_add(zs[:], pz[:],
                                                 zst[t][:, q * 1024:(q + 1) * 1024])
                            zsums.append(zs)
                    else:
                        zsums = [zst[0][:, q * 1024:(q + 1) * 1024]
                                 for q in range(4)]

                    si = lp.tile([36, 1024], F32, tag="ltmp", bufs=6, name=f"si{t}")
                    nc.scalar.activation(si[:], zsums[0][:], AF.Sigmoid)
                    sg = lp.tile([36, 1024], F32, tag="ltmp", bufs=6, name=f"sg{t}")
                    nc.scalar.activation(sg[:], zsums[2][:], AF.Tanh)
                    so = lp.tile([36, 1024], F32, tag="ltmp", bufs=6, name=f"so{t}")
                    nc.scalar.activation(so[:], zsums[3][:], AF.Sigmoid)
                    c_new = lp.tile([36, 1024], F32, tag="ltmp", bufs=6, name=f"c{t}")
                    if t > 0:
                        sf = lp.tile([36, 1024], F32, tag="ltmp", bufs=6,
                                     name=f"sf{t}")
                        nc.scalar.activation(sf[:], zsums[1][:], AF.Sigmoid)
                        t1 = lp.tile([36, 1024], F32, tag="ltmp", bufs=6,
                                     name=f"t1_{t}")
                        nc.vector.tensor_mul(t1[:], si[:], sg[:])
                        t2 = lp.tile([36, 1024], F32, tag="ltmp", bufs=6,
                                     name=f"t2_{t}")
                        nc.vector.tensor_mul(t2[:], sf[:], c_prev[:])
                        nc.vector.tensor_add(c_new[:], t1[:], t2[:])
                    else:
                        nc.vector.tensor_mul(c_new[:], si[:], sg[:])
                    c_prev = c_new
                    tch = lp.tile([36, 1024], F32, tag="ltmp", bufs=6, name=f"tc{t}")
                    nc.scalar.activation(tch[:], c_new[:], AF.Tanh)
                    ht = lp.tile([36, 1024], BF16, tag="lh", bufs=2, name=f"h{t}")
                    nc.vector.tensor_mul(ht[:], so[:], tch[:])

                    for d, base in ((0, 0), (1, 32)):
                        s = s_of[d]
                        for j in range(8):
                            tp = lps.tile([128, B], BF16, tag="ptr", bufs=2)
                            nc.tensor.transpose(
                                tp[:], ht[base:base + 4, j * 128:(j + 1) * 128],
                                ident[base:base + 4, base:base + 4])
                            nc.scalar.copy(H[d][:, j, s, :], tp[:])

                if dbg == 'hs':
                    for d in range(2):
                        nc.sync.dma_start(dbg_ap[d], H[d][:])

                # ---- ffwd 1x1 conv + leaky relu -> d0 interior
                for mc in range(2):
                    pf = lps.tile([128, 64], F32, tag="pff", bufs=2)
                    for s in range(4):
                        for kc in range(4):
                            d, chalf = kc // 2, kc % 2
                            rhs = H[d][:, chalf::2, s, :]   # [128, hh, b]
                            nc.tensor.matmul(pf[:, s * 16:(s + 1) * 16],
                                             wffsb[:, kc, mc, :], rhs,
                                             start=(kc == 0), stop=(kc == 3))
                    t1 = lp.tile([128, 64], F32, tag="ltmp", bufs=6, name=f"ff{mc}")
                    nc.scalar.activation(t1[:], pf[:], AF.Identity,
                                         bias=bias_sb[:, 16 + mc:17 + mc])
                    t2 = lp.tile([128, 64], F32, tag="ltmp", bufs=6, name=f"fm{mc}")
                    nc.vector.tensor_scalar_mul(t2[:], t1[:], 0.3)
                    dst = d0[mc][:, :, 1:5, 1:5].rearrange("p b h s -> p s h b")
                    t1v = t1.rearrange("p (s h b) -> p s h b", s=4, h=4)
                    t2v = t2.rearrange("p (s h b) -> p s h b", s=4, h=4)
                    nc.vector.tensor_max(dst, t1v, t2v)

        if dbg == 'd0':
            for mc in range(2):
                nc.sync.dma_start(dbg_ap[mc], d0[mc][:])

        # ================= decoder =================
        with tc.tile_pool(name="decp", bufs=1) as dp, \
             tc.tile_pool(name="decps", bufs=1, space="PSUM") as dpp:
            wts = []
            for l in range(4):
                wt = [dp.tile([128, 16, 2, 128], BF16, tag="dw", bufs=4,
                              name=f"wdec{l}_{kc}") for kc in range(2)]
                for kc in range(2):
                    nc.sync.dma_start(wt[kc][:], wdec[l, kc])
                wts.append(wt)

            d1 = [dp.tile([128, B, 10, 10], BF16, tag="dchain", bufs=4,
                          name=f"d1_{mc}") for mc in range(2)]
            d2 = [dp.tile([128, B, 18, 18], BF16, tag="dchain", bufs=4,
                          name=f"d2_{mc}") for mc in range(2)]
            d3 = [dp.tile([128, B, 34, 34], BF16, tag="dchain", bufs=4,
                          name=f"d3_{mc}") for mc in range(2)]
            for mc in range(2):
                memset_border(d1[mc], 10)
                memset_border(d2[mc], 18)
                memset_border(d3[mc], 34)

            def dec_layer(wt, act_in, act_out, Hin, bias_idx):
                mh_splits = 1 if B * Hin * Hin <= 512 else (B * Hin * Hin) // 512
                mrows = Hin // mh_splits
                for mc in range(2):
                    for ph in range(2):
                        for pw in range(2):
                            for mh in range(mh_splits):
                                m0 = mh * mrows
                                N = B * mrows * Hin
                                ps = dpp.tile([128, N], F32, tag="dps", bufs=6)
                                taps = [(dm, kh, dn, kw, kc)
                                        for (dm, kh) in ROW_TAPS[ph]
                                        for (dn, kw) in ROW_TAPS[pw]
                                        for kc in range(2)]
                                for i, (dm, kh, dn, kw, kc) in enumerate(taps):
                                    rhs = act_in[kc][:, :,
                                                     1 + dm + m0:1 + dm + m0 + mrows,
                                                     1 + dn:1 + dn + Hin]
                                    nc.tensor.matmul(
                                        ps[:], wt[kc][:, kh * 4 + kw, mc, :],
                                        rhs, start=(i == 0), stop=(i == 7))
                                dst = act_out[mc][:, :,
                                                  _stepped(1 + ph + 2 * m0, mrows, 2),
                                                  _stepped(1 + pw, Hin, 2)]
                                nc.scalar.activation(
                                    dst, ps[:], AF.Relu,
                                    bias=bias_sb[:, bias_idx + mc:bias_idx + mc + 1])

            dec_layer(wts[0], d0, d1, 4, 8)
            dec_layer(wts[1], d1, d2, 8, 10)
            dec_layer(wts[2], d2, d3, 16, 12)

            if dbg in ('d1', 'd2', 'd3'):
                src = {'d1': d1, 'd2': d2, 'd3': d3}[dbg]
                for mc in range(2):
                    nc.sync.dma_start(dbg_ap[mc], src[mc][:])

            # final layer + residual, streamed per (b, mc)
            for b in range(B):
                for mc in range(2):
                    xr = dp.tile([128, 64, 64], F32, tag="resid", bufs=4,
                                 name=f"xr{b}_{mc}")
                    nc.sync.dma_start(xr[:], xres[mc, :, b])
                    ob = dp.tile([128, 64, 64], F32, tag="resid", bufs=4,
                                 name=f"ob{b}_{mc}")
                    for ph in range(2):
                        for pw in range(2):
                            for mh in range(2):
                                m0 = mh * 16
                                ps = dpp.tile([128, 512], F32, tag="dps", bufs=6)
                                taps = [(dm, kh, dn, kw, kc)
                                        for (dm, kh) in ROW_TAPS[ph]
                                        for (dn, kw) in ROW_TAPS[pw]
                                        for kc in range(2)]
                                for i, (dm, kh, dn, kw, kc) in enumerate(taps):
                                    rhs = d3[kc][:, b,
                                                 1 + dm + m0:1 + dm + m0 + 16,
                                                 1 + dn:1 + dn + 32]
                                    nc.tensor.matmul(
                                        ps[:], wts[3][kc][:, kh * 4 + kw, mc, :],
                                        rhs, start=(i == 0), stop=(i == 7))
                                t1 = dp.tile([128, 512], F32, tag="fin", bufs=3,
                                             name=f"f{b}{mc}{ph}{pw}{mh}")
                                nc.scalar.activation(t1[:], ps[:], AF.Relu,
                                                     bias=bias_sb[:, 14 + mc:15 + mc])
                                oslice = ob[:, _stepped(ph + 2 * m0, 16, 2),
                                            _stepped(pw, 32, 2)]
                                xslice = xr[:, _stepped(ph + 2 * m0, 16, 2),
                                            _stepped(pw, 32, 2)]
                                t1v = t1.rearrange("p (m n) -> p m n", m=16)
                                if gamma_nonneg:
                                    nc.vector.tensor_add(oslice, t1v, xslice)
                                else:
                                    nc.vector.tensor_sub(oslice, xslice, t1v)
                    nc.sync.dma_start(out[mc, :, b], ob[:])


# --------------------------------------------------------------------------
# host-side prep + entry point
# --------------------------------------------------------------------------

def _fold_bn(w, cb, g, bb, m, v):
    A = g / np.sqrt(v + BN_EPS)
    bias = (cb - m) * A + bb
    return w * A[None, None, None, :], bias


def _pack_conv_w(w):
    # [4,4,Cin,Cout] -> [kc, ci, tap, mc, co]
    return np.ascontiguousarray(
        w.reshape(4, 4, 2, 128, 2, 128).transpose(2, 3, 0, 1, 4, 5)
        .reshape(2, 128, 16, 2, 128).astype(NPBF))


def prep_inputs(d):
    x = np.asarray(d['x'], np.float32)
    gamma = float(np.asarray(d['gamma']).reshape(-1)[0])
    g_abs, g_nonneg = abs(gamma), gamma >= 0

    wenc = np.zeros((4, 2, 128, 16, 2, 128), NPBF)
    wdec = np.zeros((4, 2, 128, 16, 2, 128), NPBF)
    bconv = np.zeros((128, 20), np.float32)
    for l in range(4):
        w, bias = _fold_bn(np.asarray(d['enc_w'][l], np.float32),
                           np.asarray(d['enc_b'][l], np.float32),
                           np.asarray(d['enc_bn_g'][l], np.float32),
                           np.asarray(d['enc_bn_b'][l], np.float32),
                           np.asarray(d['enc_bn_m'][l], np.float32),
                           np.asarray(d['enc_bn_v'][l], np.float32))
        wenc[l] = _pack_conv_w(w)
        bconv[:, l * 2] = bias[:128]
        bconv[:, l * 2 + 1] = bias[128:]
        w, bias = _fold_bn(np.asarray(d['dec_w'][l], np.float32),
                           np.asarray(d['dec_b'][l], np.float32),
                           np.asarray(d['dec_bn_g'][l], np.float32),
                           np.asarray(d['dec_bn_b'][l], np.float32),
                           np.asarray(d['dec_bn_m'][l], np.float32),
                           np.asarray(d['dec_bn_v'][l], np.float32))
        if l == 3:
            w, bias = w * g_abs, bias * g_abs
        wdec[l] = _pack_conv_w(w)
        bconv[:, 8 + l * 2] = bias[:128]
        bconv[:, 8 + l * 2 + 1] = bias[128:]
    bconv[:, 16] = np.asarray(d['ffwd_b'], np.float32)[:128]
    bconv[:, 17] = np.asarray(d['ffwd_b'], np.float32)[128:]

    wlmats = np.stack([
        np.asarray(d['lstm_fwd_W'], np.float32),
        np.asarray(d['lstm_rvs_W'], np.float32),
        np.asarray(d['lstm_fwd_U'], np.float32),
        np.asarray(d['lstm_rvs_U'], np.float32)]).reshape(4, 8, 128, 4096)
    wl = wlmats.astype(NPBF)
    blv = np.stack([np.asarray(d['lstm_fwd_b'], np.float32),
                    np.asarray(d['lstm_rvs_b'], np.float32)])
    use_bias = bool(np.any(blv != 0))
    bl = np.broadcast_to(blv[:, None, :], (2, 16, 4096)).astype(NPBF).copy()

    wffv = np.asarray(d['ffwd_w'], np.float32)[0, 0]     # [512, 256]
    wff = np.ascontiguousarray(
        wffv.reshape(4, 128, 2, 128).transpose(1, 0, 2, 3).astype(NPBF))

    xcm = np.zeros((N_CORES, 2, 128, B, 66, 66), NPBF)
    xrs = np.zeros((N_CORES, 2, 128, B, 64, 64), np.float32)
    xt = x.reshape(N_CORES, B, 64, 64, 2, 128).transpose(0, 4, 5, 1, 2, 3)
    xcm[:, :, :, :, 1:65, 1:65] = xt.astype(NPBF)
    xrs[:] = xt

    in_maps = []
    for c in range(N_CORES):
        in_maps.append(dict(xin=xcm[c], xres=xrs[c], wenc=wenc, wdec=wdec,
                            bconv=bconv, wl=wl, bl=bl, wff=wff))
    return in_maps, g_nonneg, use_bias


def get_nc(g_nonneg=True, use_bias=False, dbg=None):
    key = (g_nonneg, use_bias, dbg)
    if key not in _CACHE:
        _CACHE[key] = _build(gamma_nonneg=g_nonneg, use_bias=use_bias, dbg=dbg)
    return _CACHE[key]


def kernel(**inputs):
    in_maps, g_nonneg, use_bias = prep_inputs(inputs)
    nc = get_nc(g_nonneg, use_bias)
    res = run_bass_kernel_spmd(nc, in_maps, core_ids=list(range(N_CORES)))
    outs = []
    for c in range(N_CORES):
        o = res.results[c]["out"]          # [2, 128, B, 64, 64]
        outs.append(o.transpose(2, 3, 4, 0, 1).reshape(B, 64, 64, 256))
    return np.concatenate(outs, axis=0).astype(np.float32)


# revision 9
# speedup vs baseline: 1.2863x; 1.2863x over previous
"""Trainium2 Bass kernel for nn_AFRM_48636209660262.

Conv-BiLSTM autoencoder: 4x strided conv encoder -> channel-split BiLSTM ->
1x1 conv ffwd -> 4x conv_transpose decoder -> gamma*h + x.

Strategy: pure data parallelism over 8 NeuronCores (4 samples each, no
collectives). All activations channel-major [C_chunk(128), B, H, W] with
zero-padded spatial borders; convs are per-tap matmuls accumulated in PSUM
(lhsT = W[Cin_chunk, Cout_chunk], rhs = strided activation taps). BN scale is
folded into conv weights on the host, bias applied in the PSUM->SBUF
evacuation on the Scalar engine. conv_transpose is decomposed into 4 parity
classes x 4 taps. The LSTM runs batch-major with both directions packed into
one [36, *] tile (fwd rows 0-3, rvs rows 32-35 - both legal partition
bases); z_x is precomputed for all timesteps into [16, 4096] tiles (rvs with
reversed step order) and re-staged per round with small SBUF->SBUF DMAs; the
recurrent h@U uses h^T tiles produced by small PE transposes. Matmul
operands are bf16 (f32 PSUM accumulation); the residual add and the final
output stay f32.
"""
import numpy as np
import ml_dtypes

import concourse.mybir as mybir
import concourse.tile as tile
from concourse import bacc
from concourse.bass_utils import run_bass_kernel_spmd
from concourse.masks import make_identity

AF = mybir.ActivationFunctionType
BF16 = mybir.dt.bfloat16
F32 = mybir.dt.float32
NPBF = ml_dtypes.bfloat16

N_CORES = 8
B = 4           # batch per core
C = 256
BN_EPS = 1e-3

_CACHE: dict = {}


def _stepped(start, count, step):
    return slice(start, start + step * (count - 1) + 1, step)


def _build(gamma_nonneg=True, use_bias=False, dbg=None):
    nc = bacc.Bacc("TRN2", target_bir_lowering=False, debug=False,
                   num_devices=N_CORES)

    xin = nc.dram_tensor("xin", [2, 128, B, 66, 66], BF16, kind="ExternalInput").ap()
    xres = nc.dram_tensor("xres", [2, 128, B, 64, 64], F32, kind="ExternalInput").ap()
    wenc = nc.dram_tensor("wenc", [4, 2, 128, 16, 2, 128], BF16, kind="ExternalInput").ap()
    wdec = nc.dram_tensor("wdec", [4, 2, 128, 16, 2, 128], BF16, kind="ExternalInput").ap()
    bconv = nc.dram_tensor("bconv", [128, 20], F32, kind="ExternalInput").ap()
    # LSTM mats: [Wf, Wr, Uf, Ur] x [kc, row, 4096]
    wl = nc.dram_tensor("wl", [4, 8, 128, 4096], BF16, kind="ExternalInput").ap()
    bl = nc.dram_tensor("bl", [2, 16, 4096], BF16, kind="ExternalInput").ap()
    wff = nc.dram_tensor("wff", [128, 4, 2, 128], BF16, kind="ExternalInput").ap()
    out = nc.dram_tensor("out", [2, 128, B, 64, 64], F32, kind="ExternalOutput").ap()

    dbg_ap = None
    dbg_shapes = {
        'l1': [2, 128, B, 34, 34], 'l2': [2, 128, B, 18, 18],
        'l3': [2, 128, B, 10, 10], 'enc4': [2, 128, B * 16],
        'seqT': [128, 8, 16], 'hs': [2, 128, 8, 4, B],
        'd0': [2, 128, B, 6, 6], 'd1': [2, 128, B, 10, 10],
        'd2': [2, 128, B, 18, 18], 'd3': [2, 128, B, 34, 34],
    }
    if dbg is not None:
        dbg_ap = nc.dram_tensor("dbg", dbg_shapes[dbg], BF16,
                                kind="ExternalOutput").ap()

    with tile.TileContext(nc) as tc:
        _trace(nc, tc, xin, xres, wenc, wdec, bconv, wl, bl, wff, out,
               gamma_nonneg, use_bias, dbg, dbg_ap)
    nc.compile()
    return nc


def _trace(nc, tc, xin, xres, wenc, wdec, bconv, wl, bl, wff, out,
           gamma_nonneg, use_bias, dbg, dbg_ap):
    from contextlib import ExitStack

    def memset_border(t, Hp):
        nc.vector.memset(t[:, :, 0, :], 0.0)
        nc.vector.memset(t[:, :, Hp - 1, :], 0.0)
        nc.vector.memset(t[:, :, :, 0], 0.0)
        nc.vector.memset(t[:, :, :, Hp - 1], 0.0)

    # decoder parity taps: out[2m+p] <- pairs (di, k)
    ROW_TAPS = {0: [(-1, 0), (0, 2)], 1: [(0, 1), (1, 3)]}

    with ExitStack() as top:
        persist = top.enter_context(tc.tile_pool(name="persist", bufs=1))

        bias_sb = persist.tile([128, 20], F32)
        nc.sync.dma_start(bias_sb[:], bconv[:])
        ident = persist.tile([128, 128], BF16)
        make_identity(nc, ident[:])
        H = [persist.tile([128, 8, 4, B], BF16, name=f"hT{d}") for d in range(2)]
        enc4 = [[persist.tile([128, 4, 4], BF16, name=f"enc4_{kc}_{b}")
                 for b in range(B)] for kc in range(2)]
        d0 = [persist.tile([128, B, 6, 6], BF16, name=f"d0_{mc}") for mc in range(2)]
        wffsb = persist.tile([128, 4, 2, 128], BF16)
        nc.sync.dma_start(wffsb[:], wff[:])
        for mc in range(2):
            memset_border(d0[mc], 6)

        dram = top.enter_context(tc.tile_pool(name="dram", bufs=1, space="DRAM"))
        esc2 = [dram.tile([1024, B], BF16, name=f"esc_{sv}")
                for sv in range(4)]  # [l, b] per s

        # lwa pool spans encoder+lstm: prefetch Wf during encoder; Uf reuses
        with tc.tile_pool(name="lwa", bufs=1) as lwp:
            WLf = [lwp.tile([128, 4096], BF16, tag="lwa", bufs=8,
                            name=f"wf_{kc}") for kc in range(8)]

            # ================= encoder =================
            with tc.tile_pool(name="encp", bufs=1) as ep, \
                 tc.tile_pool(name="encps", bufs=1, space="PSUM") as pp:
                # priority order: first x slices + layer-1 weights, then the
                # rest of the encoder weights, then the LSTM Wf prefetch
                wts = []
                for l in range(4):
                    wts.append([ep.tile([128, 16, 2, 128], BF16, tag="cw",
                                        bufs=4, name=f"wenc{l}_{kc}")
                                for kc in range(2)])
                # x tiles split per (kc, b, row-half) for earliest start
                xt = [[[ep.tile([128, 34, 66], BF16, tag="xcm", bufs=16,
                                name=f"x_{kc}_{b}_{hf}") for hf in range(2)]
                       for b in range(B)] for kc in range(2)]
                for kc in range(2):
                    nc.sync.dma_start(xt[kc][0][0][:], xin[kc, :, 0, 0:34, :])
                for kc in range(2):
                    nc.sync.dma_start(wts[0][kc][:], wenc[0, kc])
                for b in range(B):
                    for hf in range(2):
                        for kc in range(2):
                            if b == 0 and hf == 0:
                                continue
                            nc.sync.dma_start(xt[kc][b][hf][:],
                                              xin[kc, :, b, 32 * hf:32 * hf + 34, :])
                for l in range(1, 4):
                    for kc in range(2):
                        nc.sync.dma_start(wts[l][kc][:], wenc[l, kc])
                for kc in range(8):
                    nc.sync.dma_start(WLf[kc][:], wl[0, kc])

                l1 = [ep.tile([128, B, 34, 34], BF16, tag="echain", bufs=4,
                              name=f"l1_{mc}") for mc in range(2)]
                l2 = [ep.tile([128, B, 18, 18], BF16, tag="echain", bufs=4,
                              name=f"l2_{mc}") for mc in range(2)]
                l3 = [ep.tile([128, B, 10, 10], BF16, tag="echain", bufs=4,
                              name=f"l3_{mc}") for mc in range(2)]
                for mc in range(2):
                    memset_border(l1[mc], 34)
                    memset_border(l2[mc], 18)
                    memset_border(l3[mc], 10)

                # L1: per (b, row-half) groups, b outer for earliest start
                for b in range(B):
                    for hf, oh0 in ((0, 0), (1, 16)):
                        for mc in range(2):
                            ps = pp.tile([128, 512], F32, tag="cps", bufs=6)
                            i = 0
                            for kc in range(2):
                                for t in range(16):
                                    kh, kw = t // 4, t % 4
                                    rhs = xt[kc][b][hf][:,
                                                        _stepped(kh, 16, 2),
                                                        _stepped(kw, 32, 2)]
                                    nc.tensor.matmul(ps[:], wts[0][kc][:, t, mc, :],
                                                     rhs, start=(i == 0), stop=(i == 31))
                                    i += 1
                            nc.scalar.activation(
                                l1[mc][:, b, 1 + oh0:17 + oh0, 1:33], ps[:],
                                AF.Relu, bias=bias_sb[:, mc:mc + 1])

                def enc_layer(wt, act_in, act_out, Hin, bias_idx):
                    OH = Hin // 2
                    groups = [(oh0, 8) for oh0 in (0, 8)] if OH == 16 else [(0, OH)]
                    for mc in range(2):
                        for (oh0, ohn) in groups:
                            ps = pp.tile([128, B * ohn * OH], F32, tag="cps", bufs=6)
                            i = 0
                            for kc in range(2):
                                for t in range(16):
                                    kh, kw = t // 4, t % 4
                                    rhs = act_in[kc][:, :,
                                                     _stepped(kh + 2 * oh0, ohn, 2),
                                                     _stepped(kw, OH, 2)]
                                    nc.tensor.matmul(ps[:], wt[kc][:, t, mc, :],
                                                     rhs, start=(i == 0), stop=(i == 31))
                                    i += 1
                            if act_out is None:
                                psv = ps.rearrange("p (b h w) -> p b h w", b=B, h=4)
                                for b in range(B):
                                    nc.scalar.activation(
                                        enc4[mc][b].rearrange("p h w -> p (h w)"),
                                        psv[:, b], AF.Relu,
                                        bias=bias_sb[:, bias_idx + mc:bias_idx + mc + 1])
                            else:
                                nc.scalar.activation(
                                    act_out[mc][:, :, 1 + oh0:1 + oh0 + ohn, 1:1 + OH],
                                    ps[:], AF.Relu,
                                    bias=bias_sb[:, bias_idx + mc:bias_idx + mc + 1])

                enc_layer(wts[1], l1, l2, 32, 2)
                enc_layer(wts[2], l2, l3, 16, 4)
                enc_layer(wts[3], l3, None, 8, 6)

                if dbg in ('l1', 'l2', 'l3'):
                    src = {'l1': l1, 'l2': l2, 'l3': l3}[dbg]
                    for mc in range(2):
                        nc.sync.dma_start(dbg_ap[mc], src[mc][:])

            # enc4 -> l-major DRAM bounce: l = hh*256 + ww*64 + cc, s=2kc+shi
            # each s-chain runs on its own engine queue to parallelize
            dmaeng = [nc.gpsimd, nc.scalar, nc.sync, nc.gpsimd]
            for kc in range(2):
                for shi in range(2):
                    sv = 2 * kc + shi
                    for b in range(B):
                        src = enc4[kc][b][shi * 64:(shi + 1) * 64, :, :] \
                            .rearrange("cc hh ww -> cc (hh ww)")
                        dst = esc2[sv].rearrange("(hw cc) b -> cc hw b",
                                                 cc=64)[:, :, b]
                        dmaeng[sv].dma_start(dst, src)
            if dbg == 'enc4':
                for kc in range(2):
                    for b in range(B):
                        nc.sync.dma_start(
                            dbg_ap[kc, :, b * 16:(b + 1) * 16],
                            enc4[kc][b].rearrange("p h w -> p (h w)"))

            # ================= LSTM =================
            with tc.tile_pool(name="lstmp", bufs=1) as lp, \
                 tc.tile_pool(name="lstmps", bufs=1, space="PSUM") as lps:
                # seqT [p, j, (s,b)] straight from the l-major bounce
                seqT = lp.tile([128, 8, 16], BF16)
                seqTr = lp.tile([128, 8, 16], BF16)
                dmaeng = [nc.gpsimd, nc.scalar, nc.sync, nc.gpsimd]
                for sv in range(4):
                    src = esc2[sv].rearrange("(j p) b -> p j b", j=8)
                    dmaeng[sv].dma_start(seqT[:, :, sv * 4:(sv + 1) * 4], src)
                    dmaeng[sv].dma_start(
                        seqTr[:, :, (3 - sv) * 4:(4 - sv) * 4], src)
                if dbg == 'seqT':
                    nc.sync.dma_start(dbg_ap[:], seqT[:])

                WLr = [lp.tile([128, 4096], BF16, tag="lwb", bufs=8,
                               name=f"wr_{kc}") for kc in range(8)]
                for kc in range(8):
                    nc.sync.dma_start(WLr[kc][:], wl[1, kc])

                # ---- z_x for all steps; rvs rows are step-reversed so that
                # round t always reads rows [4t, 4t+4)
                zxe = []
                for d, sq, WT in ((0, seqT, WLf), (1, seqTr, WLr)):
                    blt = None
                    if use_bias:
                        blt = lp.tile([16, 4096], BF16, tag="zxj", bufs=4,
                                      name=f"bl{d}")
                        nc.sync.dma_start(blt[:], bl[d])
                    zx = lp.tile([16, 4096], BF16, tag="zxj", bufs=4,
                                 name=f"zxe{d}")
                    for q in range(4):
                        ps = lps.tile([16, 1024], F32, tag="pz", bufs=2)
                        for kc in range(8):
                            for nb in range(2):
                                nc.tensor.matmul(
                                    ps[:, nb * 512:(nb + 1) * 512],
                                    sq[:, kc, :],
                                    WT[kc][:, q * 1024 + nb * 512:
                                           q * 1024 + (nb + 1) * 512],
                                    start=(kc == 0), stop=(kc == 7))
                        if use_bias:
                            bsl = blt[:, q * 1024:(q + 1) * 1024]
                            if d == 1:  # bias rows follow the reversed order too
                                bsl = blt[:, q * 1024:(q + 1) * 1024]
                            nc.vector.tensor_add(zx[:, q * 1024:(q + 1) * 1024],
                                                 ps[:], bsl)
                        else:
                            nc.vector.tensor_copy(zx[:, q * 1024:(q + 1) * 1024],
                                                  ps[:])
                    zxe.append(zx)

                # ---- stage all rounds' z slices early (SBUF->SBUF)
                zst = []
                stage_eng = [nc.gpsimd, nc.scalar, nc.sync, nc.gpsimd]
                for t in range(4):
                    z = lp.tile([36, 4096], BF16, tag="zxj", bufs=4,
                                name=f"zst{t}")
                    stage_eng[t].dma_start(z[0:4, :], zxe[0][4 * t:4 * t + 4, :])
                    stage_eng[t].dma_start(z[32:36, :], zxe[1][4 * t:4 * t + 4, :])
                    zst.append(z)

                # U matrices reuse the W slots
                ULf = [lwp.tile([128, 4096], BF16, tag="lwa", bufs=8,
                                name=f"uf_{kc}") for kc in range(8)]
                for kc in range(8):
                    nc.sync.dma_start(ULf[kc][:], wl[2, kc])
                ULr = [lp.tile([128, 4096], BF16, tag="lwb", bufs=8,
                               name=f"ur_{kc}") for kc in range(8)]
                for kc in range(8):
                    nc.sync.dma_start(ULr[kc][:], wl[3, kc])
                UL = [ULf, ULr]

                # ---- recurrence: joint [36, *], fwd rows 0-3, rvs rows 32-35
                c_prev = None
                for t in range(4):
                    s_of = {0: t, 1: 3 - t}
                    zsums = []
                    if t > 0:
                        for q in range(4):
                            pz = lps.tile([36, 1024], F32, tag="pz", bufs=2)
                            for d, base in ((0, 0), (1, 32)):
                                sp = s_of[d] + (1 if d else -1)
                                for kc in range(8):
                                    for nb in range(2):
                                        nc.tensor.matmul(
                                            pz[base:base + 4,
                                               nb * 512:(nb + 1) * 512],
                                            H[d][:, kc, sp, :],
                                            UL[d][kc][:, q * 1024 + nb * 512:
                                                      q * 1024 + (nb + 1) * 512],
                                            start=(kc == 0), stop=(kc == 7))
                            zs = lp.tile([36, 1024], F32, tag="ltmp", bufs=6,
                                         name=f"zs{t}_{q}")
                            nc.vector.tensor_add(zs[:], pz[:],
                                                 zst[t][:, q * 1024:(q + 1) * 1024])
                            zsums.append(zs)
                    else:
                        zsums = [zst[0][:, q * 1024:(q + 1) * 1024]
                                 for q in range(4)]

                    si = lp.tile([36, 1024], F32, tag="ltmp", bufs=6, name=f"si{t}")
                    nc.scalar.activation(si[:], zsums[0][:], AF.Sigmoid)
                    sg = lp.tile([36, 1024], F32, tag="ltmp", bufs=6, name=f"sg{t}")
                    nc.scalar.activation(sg[:], zsums[2][:], AF.Tanh)
                    so = lp.tile([36, 1024], F32, tag="ltmp", bufs=6, name=f"so{t}")
                    nc.scalar.activation(so[:], zsums[3][:], AF.Sigmoid)
                    c_new = lp.tile([36, 1024], F32, tag="ltmp", bufs=6, name=f"c{t}")
                    if t > 0:
                        sf = lp.tile([36, 1024], F32, tag="ltmp", bufs=6,
                                     name=f"sf{t}")
                        nc.scalar.activation(sf[:], zsums[1][:], AF.Sigmoid)
                        t1 = lp.tile([36, 1024], F32, tag="ltmp", bufs=6,
                                     name=f"t1_{t}")
                        nc.vector.tensor_mul(t1[:], si[:], sg[:])
                        t2 = lp.tile([36, 1024], F32, tag="ltmp", bufs=6,
                                     name=f"t2_{t}")
                        nc.vector.tensor_mul(t2[:], sf[:], c_prev[:])
                        nc.vector.tensor_add(c_new[:], t1[:], t2[:])
                    else:
                        nc.vector.tensor_mul(c_new[:], si[:], sg[:])
                    c_prev = c_new
                    tch = lp.tile([36, 1024], F32, tag="ltmp", bufs=6, name=f"tc{t}")
                    nc.scalar.activation(tch[:], c_new[:], AF.Tanh)
                    ht = lp.tile([36, 1024], BF16, tag="lh", bufs=2, name=f"h{t}")
                    nc.vector.tensor_mul(ht[:], so[:], tch[:])

                    for d, base in ((0, 0), (1, 32)):
                        s = s_of[d]
                        for j in range(8):
                            tp = lps.tile([128, B], BF16, tag="ptr", bufs=2)
                            nc.tensor.transpose(
                                tp[:], ht[base:base + 4, j * 128:(j + 1) * 128],
                                ident[base:base + 4, base:base + 4])
                            nc.scalar.copy(H[d][:, j, s, :], tp[:])

                if dbg == 'hs':
                    for d in range(2):
                        nc.sync.dma_start(dbg_ap[d], H[d][:])

                # ---- ffwd 1x1 conv + leaky relu -> d0 interior
                for mc in range(2):
                    pf = lps.tile([128, 64], F32, tag="pff", bufs=2)
                    for s in range(4):
                        for kc in range(4):
                            d, chalf = kc // 2, kc % 2
                            rhs = H[d][:, chalf::2, s, :]   # [128, hh, b]
                            nc.tensor.matmul(pf[:, s * 16:(s + 1) * 16],
                                             wffsb[:, kc, mc, :], rhs,
                                             start=(kc == 0), stop=(kc == 3))
                    t1 = lp.tile([128, 64], F32, tag="ltmp", bufs=6, name=f"ff{mc}")
                    nc.scalar.activation(t1[:], pf[:], AF.Identity,
                                         bias=bias_sb[:, 16 + mc:17 + mc])
                    t2 = lp.tile([128, 64], F32, tag="ltmp", bufs=6, name=f"fm{mc}")
                    nc.vector.tensor_scalar_mul(t2[:], t1[:], 0.3)
                    dst = d0[mc][:, :, 1:5, 1:5].rearrange("p b h s -> p s h b")
                    t1v = t1.rearrange("p (s h b) -> p s h b", s=4, h=4)
                    t2v = t2.rearrange("p (s h b) -> p s h b", s=4, h=4)
                    nc.vector.tensor_max(dst, t1v, t2v)

        if dbg == 'd0':
            for mc in range(2):
                nc.sync.dma_start(dbg_ap[mc], d0[mc][:])

        # ================= decoder =================
        with tc.tile_pool(name="decp", bufs=1) as dp, \
             tc.tile_pool(name="decps", bufs=1, space="PSUM") as dpp:
            wts = []
            for l in range(4):
                wt = [dp.tile([128, 16, 2, 128], BF16, tag="dw", bufs=4,
                              name=f"wdec{l}_{kc}") for kc in range(2)]
                for kc in range(2):
                    nc.sync.dma_start(wt[kc][:], wdec[l, kc])
                wts.append(wt)

            d1 = [dp.tile([128, B, 10, 10], BF16, tag="dchain", bufs=4,
                          name=f"d1_{mc}") for mc in range(2)]
            d2 = [dp.tile([128, B, 18, 18], BF16, tag="dchain", bufs=4,
                          name=f"d2_{mc}") for mc in range(2)]
            d3 = [dp.tile([128, B, 34, 34], BF16, tag="dchain", bufs=4,
                          name=f"d3_{mc}") for mc in range(2)]
            for mc in range(2):
                memset_border(d1[mc], 10)
                memset_border(d2[mc], 18)
                memset_border(d3[mc], 34)

            def dec_layer(wt, act_in, act_out, Hin, bias_idx):
                mh_splits = 1 if B * Hin * Hin <= 512 else (B * Hin * Hin) // 512
                mrows = Hin // mh_splits
                for mc in range(2):
                    for ph in range(2):
                        for pw in range(2):
                            for mh in range(mh_splits):
                                m0 = mh * mrows
                                N = B * mrows * Hin
                                ps = dpp.tile([128, N], F32, tag="dps", bufs=6)
                                taps = [(dm, kh, dn, kw, kc)
                                        for (dm, kh) in ROW_TAPS[ph]
                                        for (dn, kw) in ROW_TAPS[pw]
                                        for kc in range(2)]
                                for i, (dm, kh, dn, kw, kc) in enumerate(taps):
                                    rhs = act_in[kc][:, :,
                                                     1 + dm + m0:1 + dm + m0 + mrows,
                                                     1 + dn:1 + dn + Hin]
                                    nc.tensor.matmul(
                                        ps[:], wt[kc][:, kh * 4 + kw, mc, :],
                                        rhs, start=(i == 0), stop=(i == 7))
                                dst = act_out[mc][:, :,
                                                  _stepped(1 + ph + 2 * m0, mrows, 2),
                                                  _stepped(1 + pw, Hin, 2)]
                                nc.scalar.activation(
                                    dst, ps[:], AF.Relu,
                                    bias=bias_sb[:, bias_idx + mc:bias_idx + mc + 1])

            dec_layer(wts[0], d0, d1, 4, 8)
            dec_layer(wts[1], d1, d2, 8, 10)
            dec_layer(wts[2], d2, d3, 16, 12)

            if dbg in ('d1', 'd2', 'd3'):
                src = {'d1': d1, 'd2': d2, 'd3': d3}[dbg]
                for mc in range(2):
                    nc.sync.dma_start(dbg_ap[mc], src[mc][:])

            # final layer + residual, streamed per (b, mc)
            for b in range(B):
                for mc in range(2):
                    xr = dp.tile([128, 64, 64], F32, tag="resid", bufs=4,
                                 name=f"xr{b}_{mc}")
                    nc.sync.dma_start(xr[:], xres[mc, :, b])
                    ob = dp.tile([128, 64, 64], F32, tag="resid", bufs=4,
                                 name=f"ob{b}_{mc}")
                    for ph in range(2):
                        for pw in range(2):
                            for mh in range(2):
                                m0 = mh * 16
                                ps = dpp.tile([128, 512], F32, tag="dps", bufs=6)
                                taps = [(dm, kh, dn, kw, kc)
                                        for (dm, kh) in ROW_TAPS[ph]
                                        for (dn, kw) in ROW_TAPS[pw]
                                        for kc in range(2)]
                                for i, (dm, kh, dn, kw, kc) in enumerate(taps):
                                    rhs = d3[kc][:, b,
                                                 1 + dm + m0:1 + dm + m0 + 16,
                                                 1 + dn:1 + dn + 32]
                                    nc.tensor.matmul(
                                        ps[:], wts[3][kc][:, kh * 4 + kw, mc, :],
                                        rhs, start=(i == 0), stop=(i == 7))
                                t1 = dp.tile([128, 512], F32, tag="fin", bufs=3,
                                             name=f"f{b}{mc}{ph}{pw}{mh}")
                                nc.scalar.activation(t1[:], ps[:], AF.Relu,
                                                     bias=bias_sb[:, 14 + mc:15 + mc])
                                oslice = ob[:, _stepped(ph + 2 * m0, 16, 2),
                                            _stepped(pw, 32, 2)]
                                xslice = xr[:, _stepped(ph + 2 * m0, 16, 2),
                                            _stepped(pw, 32, 2)]
                                t1v = t1.rearrange("p (m n) -> p m n", m=16)
                                if gamma_nonneg:
                                    nc.vector.tensor_add(oslice, t1v, xslice)
                                else:
                                    nc.vector.tensor_sub(oslice, xslice, t1v)
                    nc.sync.dma_start(out[mc, :, b], ob[:])


# --------------------------------------------------------------------------
# host-side prep + entry point
# --------------------------------------------------------------------------

def _fold_bn(w, cb, g, bb, m, v):
    A = g / np.sqrt(v + BN_EPS)
    bias = (cb - m) * A + bb
    return w * A[None, None, None, :], bias


def _pack_conv_w(w):
    # [4,4,Cin,Cout] -> [kc, ci, tap, mc, co]
    return np.ascontiguousarray(
        w.reshape(4, 4, 2, 128, 2, 128).transpose(2, 3, 0, 1, 4, 5)
        .reshape(2, 128, 16, 2, 128).astype(NPBF))


def prep_inputs(d):
    x = np.asarray(d['x'], np.float32)
    gamma = float(np.asarray(d['gamma']).reshape(-1)[0])
    g_abs, g_nonneg = abs(gamma), gamma >= 0

    wenc = np.zeros((4, 2, 128, 16, 2, 128), NPBF)
    wdec = np.zeros((4, 2, 128, 16, 2, 128), NPBF)
    bconv = np.zeros((128, 20), np.float32)
    for l in range(4):
        w, bias = _fold_bn(np.asarray(d['enc_w'][l], np.float32),
                           np.asarray(d['enc_b'][l], np.float32),
                           np.asarray(d['enc_bn_g'][l], np.float32),
                           np.asarray(d['enc_bn_b'][l], np.float32),
                           np.asarray(d['enc_bn_m'][l], np.float32),
                           np.asarray(d['enc_bn_v'][l], np.float32))
        wenc[l] = _pack_conv_w(w)
        bconv[:, l * 2] = bias[:128]
        bconv[:, l * 2 + 1] = bias[128:]
        w, bias = _fold_bn(np.asarray(d['dec_w'][l], np.float32),
                           np.asarray(d['dec_b'][l], np.float32),
                           np.asarray(d['dec_bn_g'][l], np.float32),
                           np.asarray(d['dec_bn_b'][l], np.float32),
                           np.asarray(d['dec_bn_m'][l], np.float32),
                           np.asarray(d['dec_bn_v'][l], np.float32))
        if l == 3:
            w, bias = w * g_abs, bias * g_abs
        wdec[l] = _pack_conv_w(w)
        bconv[:, 8 + l * 2] = bias[:128]
        bconv[:, 8 + l * 2 + 1] = bias[128:]
    bconv[:, 16] = np.asarray(d['ffwd_b'], np.float32)[:128]
    bconv[:, 17] = np.asarray(d['ffwd_b'], np.float32)[128:]

    wlmats = np.stack([
        np.asarray(d['lstm_fwd_W'], np.float32),
        np.asarray(d['lstm_rvs_W'], np.float32),
        np.asarray(d['lstm_fwd_U'], np.float32),
        np.asarray(d['lstm_rvs_U'], np.float32)]).reshape(4, 8, 128, 4096)
    wl = wlmats.astype(NPBF)
    blv = np.stack([np.asarray(d['lstm_fwd_b'], np.float32),
                    np.asarray(d['lstm_rvs_b'], np.float32)])
    use_bias = bool(np.any(blv != 0))
    bl = np.broadcast_to(blv[:, None, :], (2, 16, 4096)).astype(NPBF).copy()

    wffv = np.asarray(d['ffwd_w'], np.float32)[0, 0]     # [512, 256]
    wff = np.ascontiguousarray(
        wffv.reshape(4, 128, 2, 128).transpose(1, 0, 2, 3).astype(NPBF))

    xcm = np.zeros((N_CORES, 2, 128, B, 66, 66), NPBF)
    xrs = np.zeros((N_CORES, 2, 128, B, 64, 64), np.float32)
    xt = x.reshape(N_CORES, B, 64, 64, 2, 128).transpose(0, 4, 5, 1, 2, 3)
    xcm[:, :, :, :, 1:65, 1:65] = xt.astype(NPBF)
    xrs[:] = xt

    in_maps = []
    for c in range(N_CORES):
        in_maps.append(dict(xin=xcm[c], xres=xrs[c], wenc=wenc, wdec=wdec,
                            bconv=bconv, wl=wl, bl=bl, wff=wff))
    return in_maps, g_nonneg, use_bias


def get_nc(g_nonneg=True, use_bias=False, dbg=None):
    key = (g_nonneg, use_bias, dbg)
    if key not in _CACHE:
        _CACHE[key] = _build(gamma_nonneg=g_nonneg, use_bias=use_bias, dbg=dbg)
    return _CACHE[key]


def kernel(**inputs):
    in_maps, g_nonneg, use_bias = prep_inputs(inputs)
    nc = get_nc(g_nonneg, use_bias)
    res = run_bass_kernel_spmd(nc, in_maps, core_ids=list(range(N_CORES)))
    outs = []
    for c in range(N_CORES):
        o = res.results[c]["out"]          # [2, 128, B, 64, 64]
        outs.append(o.transpose(2, 3, 4, 0, 1).reshape(B, 64, 64, 256))
    return np.concatenate(outs, axis=0).astype(np.float32)
